# revision 1
# baseline (speedup 1.0000x reference)
"""Trainium2 Bass kernel for a dense transformer block (causal attn + MLP).

Problem: B=4, L=2048, D=1024, H=16 (DH=64), DFF=4096, fp32 in/out.

Sharding: 8 cores = 4 batches x 2 parity groups. Core c handles batch
b=c//2 and query-row tiles {p, p+2, ..., p+14} (p=c%2); interleaved
128-row tiles balance causal-attention work between the two cores of a
batch.

v2 design (vs v1 baseline):
- The host passes X pre-transposed, own-parity columns only (xto). All
  LayerNorms run in the transposed (feature-major) layout via ones-row
  matmul statistics; there are NO PE transposes anywhere.
- K/V are computed only for the core's own-parity key tiles (half the
  sequence) and exchanged with the pair core through an HBM AllGather
  (replica groups [[0,1],[2,3],[4,5],[6,7]]).
- Attention runs Bk-outer with a two-deep score->exp->AV software
  pipeline; causal trimming is exact to the 128-column chunk and the
  boundary needs only a 128-wide mask (universal triangle / ones /
  zeros by parity).
- Softmax normalization is deferred through AV via a ones column in V;
  1/sumexp is broadcast with a rank-1 PE outer product (norm_tail),
  emitted one block late so its latency never stalls the PE.
- AV stays in SBUF (av_sb); the WO projection is fused into the
  attention stream (one output tile per ht at block boundaries). The
  post-attention residual x2 lives in DRAM (SBUF pressure), re-read by
  LN2 and the MLP residual add.
- Matmul operand dtypes are uniform per matmul: bf16 for projections /
  attention / MLP, f32r (full-rate fp32) for LN statistics.
"""

import numpy as np
import ml_dtypes

import concourse.bacc as bacc
import concourse.bass as bass
import concourse.mybir as mybir
import concourse.tile as tile
from concourse.bass_utils import run_bass_kernel_spmd

F32 = mybir.dt.float32
F32R = mybir.dt.float32r
BF16 = mybir.dt.bfloat16
BF = ml_dtypes.bfloat16
EPS = 1e-5
AF = mybir.ActivationFunctionType
OP = mybir.AluOpType

B_, L_, D_, H_, DFF_ = 4, 2048, 1024, 16, 4096
N_CORES = 8


def _ja(i, Bk):
    """First computed 128-col own chunk for key tile i in query block Bk
    (parity-shared; exact for p=1, one wasted-then-zeroed chunk for p=0
    on odd offsets)."""
    return min(3, max(0, (i - 8 * Bk) // 2))


def _derived(L, D, H, DFF):
    CT = D // 128
    FT = DFF // 128
    n_lt = L // 128
    n_own = n_lt // 2
    OWN_L = n_own * 128
    NB = OWN_L // 512
    HT = H // 2
    assert CT == HT
    return dict(CT=CT, FT=FT, n_lt=n_lt, n_own=n_own, OWN_L=OWN_L, NB=NB,
                HT=HT)


def build_nc(L=L_, D=D_, H=H_, DFF=DFF_, n_cores=N_CORES):
    g = _derived(L, D, H, DFF)
    CT, FT = g["CT"], g["FT"]
    n_lt, n_own, OWN_L, NB, HT = (g["n_lt"], g["n_own"], g["OWN_L"],
                                  g["NB"], g["HT"])
    W = 512
    NBLK = OWN_L // W          # own-column 512-blocks (= NB)
    OST = n_own // NBLK        # own seq tiles per 512-block (4)
    scale = 1.0 / 8.0          # 1/sqrt(DH)
    KT_SZ = CT * OWN_L         # kt section cols in exchange buffer
    V_SZ = n_own * H * 65      # v section cols
    rg = [[2 * b, 2 * b + 1] for b in range(n_cores // 2)]

    nc = bacc.Bacc("TRN2", target_bir_lowering=False, debug=False,
                   num_devices=n_cores)

    dp = nc.declare_dram_parameter
    xto_d = dp("xto", [128, CT, OWN_L], F32, isOutput=False)
    wq_d = dp("wq", [128, CT, CT, 128], BF16, isOutput=False)  # [p,d,c,q]
    wk_d = dp("wk", [128, CT, CT, 128], BF16, isOutput=False)
    wv_d = dp("wv", [128, CT, D], BF16, isOutput=False)        # [p,c,dv]
    wo_d = dp("wo", [CT, 128, CT, 128], BF16, isOutput=False)  # [e,p,c,q]
    w1_d = dp("w1", [FT, 128, CT, 128], BF16, isOutput=False)  # [f,p,c,q]
    w2_d = dp("w2", [CT, 128, FT, 128], BF16, isOutput=False)  # [e,p,f,q]
    bq_d = dp("bqc", [128, CT], F32, isOutput=False)
    bk_d = dp("bkc", [128, CT], F32, isOutput=False)
    b1_d = dp("b1c", [128, FT], F32, isOutput=False)
    boeff_d = dp("boeffc", [128, CT], F32, isOutput=False)
    b2_d = dp("b2c", [128, CT], F32, isOutput=False)
    onescv_d = dp("onescv", [128, 1], F32, isOutput=False)
    onesrv_d = dp("onesrv", [1, 128], F32, isOutput=False)
    masks_d = dp("masks", [128, 2, 128], BF16, isOutput=False)
    out_d = dp("outT", [D, OWN_L], F32, isOutput=True)
    # per-512-block exchange buffers: [blk][128, kt-half + v-half]
    HKT = CT * W              # kt cols for one block's 4 own tiles
    HV = (n_own // NBLK) * H * 65
    kvs_d = [nc.dram_tensor(f"kvs{b}_dram", [128, HKT + HV], BF16)
             for b in range(NBLK)]
    kvr_d = [nc.dram_tensor(f"kvr{b}_dram", [2, 128, HKT + HV], BF16)
             for b in range(NBLK)]
    x2_d = nc.dram_tensor("x2_dram", [D, OWN_L], F32R)

    with tile.TileContext(nc) as tc, \
         nc.allow_low_precision(reason="bf16/f32r matmul operands by design"):
        consts_cm = tc.tile_pool(name="consts", bufs=1)
        consts = consts_cm.__enter__()

        eps_c = consts.tile([128, 1], F32, tag="eps")
        nc.vector.memset(eps_c[:], EPS)
        ones_c = consts.tile([128, 1], F32R, tag="onesc")
        nc.sync.dma_start(out=ones_c[:], in_=onescv_d[:].bitcast(F32R))
        ones_r = consts.tile([1, 128], F32R, tag="onesr")
        nc.sync.dma_start(out=ones_r[:], in_=onesrv_d[:].bitcast(F32R))
        _oap = onesrv_d[:]
        onesm = consts.tile([128, 128], F32R, tag="onesm")
        nc.sync.dma_start(out=onesm[:], in_=bass.AP(
            tensor=_oap.tensor, offset=_oap.offset,
            ap=[[0, 128], [1, 128]]).bitcast(F32R))
        bq_sb = consts.tile([128, CT], F32, tag="bq")
        nc.sync.dma_start(out=bq_sb[:], in_=bq_d[:])
        bk_sb = consts.tile([128, CT], F32, tag="bk")
        nc.sync.dma_start(out=bk_sb[:], in_=bk_d[:])
        b1_sb = consts.tile([128, FT], F32, tag="b1")
        nc.sync.dma_start(out=b1_sb[:], in_=b1_d[:])
        boeff_sb = consts.tile([128, CT], F32, tag="boeff")
        nc.sync.dma_start(out=boeff_sb[:], in_=boeff_d[:])
        b2_sb = consts.tile([128, CT], F32, tag="b2")
        nc.sync.dma_start(out=b2_sb[:], in_=b2_d[:])
        masks_sb = consts.tile([128, 2, 128], BF16, tag="masks")
        nc.sync.dma_start(out=masks_sb[:], in_=masks_d[:])

        # long-lived SBUF state (open before phase-local pools; LIFO)
        x2nt_cm = tc.tile_pool(name="x2nt", bufs=1)
        p_x2nt = x2nt_cm.__enter__()
        x2nt = [p_x2nt.tile([128, OWN_L], BF16, tag=f"x2nt{i}",
                            name=f"x2nt{i}") for i in range(CT)]
        attio_cm = tc.tile_pool(name="attio", bufs=1)
        attio = attio_cm.__enter__()
        kt = [attio.tile([128, L], BF16, tag=f"kt{i}", name=f"kt{i}")
              for i in range(CT)]
        qt = [attio.tile([128, OWN_L], BF16, tag=f"qt{i}", name=f"qt{i}")
              for i in range(CT)]
        v_sb = [attio.tile([128, H, 65], BF16, tag=f"v{i}", name=f"v{i}")
                for i in range(n_lt)]
        av_cm = tc.tile_pool(name="avp", bufs=1)
        p_av = av_cm.__enter__()
        av_sb = [p_av.tile([128, OWN_L], BF16, tag=f"av{i}", name=f"av{i}")
                 for i in range(CT)]

        # ============ Phase A: LN1 (transposed) + QKV + exchange ========
        wqkv_cm = tc.tile_pool(name="wqkv", bufs=1)
        wqkv = wqkv_cm.__enter__()
        xt_cm = tc.tile_pool(name="xtp", bufs=1)
        p_xt = xt_cm.__enter__()
        xn_cm = tc.tile_pool(name="xnp", bufs=1)
        p_xn = xn_cm.__enter__()
        wA_cm = tc.tile_pool(name="workA", bufs=2)
        wA = wA_cm.__enter__()
        psA_st_cm = tc.tile_pool(name="psA_st", bufs=2, space="PSUM")
        psA_st = psA_st_cm.__enter__()
        psA_mm_cm = tc.tile_pool(name="psA_mm", bufs=4, space="PSUM")
        psA_mm = psA_mm_cm.__enter__()

        wv_sb = wqkv.tile([128, CT, D], BF16, tag="wv", name="wv_sb")
        nc.sync.dma_start(out=wv_sb[:], in_=wv_d[:])

        xn = [[None] * CT for _ in range(NBLK)]
        for blk in range(NBLK):
            xt = [p_xt.tile([128, W], F32R, tag=f"xt{ci}", name=f"xt{ci}")
                  for ci in range(CT)]
            for ci in range(CT):
                nc.sync.dma_start(
                    out=xt[ci][:],
                    in_=xto_d[:, ci, blk * W:(blk + 1) * W].bitcast(F32R))
            # prefetch all squares so the PE never ping-pongs with ACT
            sqs = []
            for ci in range(CT):
                sq = wA.tile([128, W], F32R, tag="sq", bufs=CT)
                nc.scalar.activation(out=sq[:], in_=xt[ci][:],
                                     func=AF.Square)
                sqs.append(sq)
            ps_mu = psA_st.tile([1, W], F32, tag="ps_mu")
            ps_sq = psA_st.tile([1, W], F32, tag="ps_sq")
            for ci in range(CT):
                nc.tensor.matmul(ps_mu[:], ones_c[:], xt[ci][:],
                                 start=(ci == 0), stop=(ci == CT - 1))
            for ci in range(CT):
                nc.tensor.matmul(ps_sq[:], ones_c[:], sqs[ci][:],
                                 start=(ci == 0), stop=(ci == CT - 1))
            mur = wA.tile([1, W], F32R, tag="mur", bufs=1)
            nc.vector.tensor_scalar_mul(out=mur[:], in0=ps_mu[:],
                                        scalar1=1.0 / D)
            mu2 = wA.tile([1, W], F32, tag="mu2", bufs=1)
            nc.vector.tensor_mul(mu2[:], mur[:], mur[:])
            varr = wA.tile([1, W], F32, tag="varr", bufs=1)
            nc.vector.tensor_scalar_mul(out=varr[:], in0=ps_sq[:],
                                        scalar1=1.0 / D)
            nc.vector.tensor_sub(varr[:], varr[:], mu2[:])
            stdr = wA.tile([1, W], F32, tag="stdr", bufs=1)
            nc.scalar.activation(out=stdr[:], in_=varr[:], func=AF.Sqrt,
                                 bias=eps_c[0:1, :])
            rstdf = wA.tile([1, W], F32, tag="rstdf", bufs=1)
            nc.vector.reciprocal_approx_fast(out=rstdf[:], in_=stdr[:])
            rstdr = wA.tile([1, W], F32R, tag="rstdr", bufs=1)
            nc.vector.tensor_copy(out=rstdr[:], in_=rstdf[:])
            ps_mub = psA_mm.tile([128, W], F32, tag="ps_mm", name="ps_mub")
            nc.tensor.matmul(ps_mub[:], ones_r[:], mur[:],
                             start=True, stop=True)
            ps_rsb = psA_mm.tile([128, W], F32, tag="ps_mm", name="ps_rsb")
            nc.tensor.matmul(ps_rsb[:], ones_r[:], rstdr[:],
                             start=True, stop=True)
            for ci in range(CT):
                t1 = wA.tile([128, W], F32, tag="t1")
                nc.vector.tensor_sub(t1[:], xt[ci][:], ps_mub[:])
                xnt = p_xn.tile([128, W], BF16, tag=f"xn{blk}_{ci}",
                                name=f"xn{blk}_{ci}")
                nc.vector.tensor_mul(xnt[:], t1[:], ps_rsb[:])
                xn[blk][ci] = xnt

            # K for this block's own-parity key tiles (wk streamed)
            for di in range(CT):
                wkt = wqkv.tile([128, CT, 128], BF16, tag="wqk",
                                name="wkt", bufs=2)
                nc.sync.dma_start(out=wkt[:], in_=wk_d[:, di])
                ps = psA_mm.tile([128, W], F32, tag="ps_mm", name="ps_k")
                for ci in range(CT):
                    nc.tensor.matmul(ps[:], wkt[:, ci, :],
                                     xn[blk][ci][:],
                                     start=(ci == 0), stop=(ci == CT - 1))
                kst = wA.tile([128, W], BF16, tag="kst")
                nc.vector.tensor_scalar_add(out=kst[:], in0=ps[:],
                                            scalar1=bk_sb[:, di:di + 1])
                nc.sync.dma_start(out=kvs_d[blk][:, di * W:(di + 1) * W],
                                  in_=kst[:])
            # V for this block's own-parity key tiles
            for st4 in range(OST):
                vst = wA.tile([128, H, 65], BF16, tag="vst")
                nc.vector.memset(vst[:], 1.0)
                for vb in range(D // W):
                    ps = psA_mm.tile([128, W], F32, tag="ps_mm", name="ps_v")
                    for ci in range(CT):
                        nc.tensor.matmul(
                            ps[:], xn[blk][ci][:, st4 * 128:(st4 + 1) * 128],
                            wv_sb[:, ci, vb * W:(vb + 1) * W],
                            start=(ci == 0), stop=(ci == CT - 1))
                    nhh = W // 64
                    nc.vector.tensor_copy(
                        out=vst[:, vb * nhh:(vb + 1) * nhh, 0:64],
                        in_=ps[:].rearrange("p (h d) -> p h d", d=64))
                nc.sync.dma_start(
                    out=kvs_d[blk][:, HKT + st4 * (H * 65):
                                   HKT + (st4 + 1) * (H * 65)].rearrange(
                                       "p (h e) -> p h e", e=65),
                    in_=vst[:])
            # exchange this block's K/V with the pair core (overlaps the
            # rest of phase A)
            nc.gpsimd.collective_compute(
                "AllGather", mybir.AluOpType.bypass, replica_groups=rg,
                ins=[kvs_d[blk][:].opt()], outs=[kvr_d[blk][:].opt()])

        # Q for own columns (overlaps the collectives on PE)
        for blk in range(NBLK):
            for di in range(CT):
                wqt = wqkv.tile([128, CT, 128], BF16, tag="wqk",
                                name="wqt", bufs=2)
                nc.sync.dma_start(out=wqt[:], in_=wq_d[:, di])
                ps = psA_mm.tile([128, W], F32, tag="ps_mm", name="ps_q")
                for ci in range(CT):
                    nc.tensor.matmul(ps[:], wqt[:, ci, :],
                                     xn[blk][ci][:],
                                     start=(ci == 0), stop=(ci == CT - 1))
                nc.vector.tensor_scalar_add(
                    out=qt[di][:, blk * W:(blk + 1) * W], in0=ps[:],
                    scalar1=bq_sb[:, di:di + 1])

        # scatter exchanged K/V into global-indexed SBUF tiles
        for blk in range(NBLK):
            for r in range(2):
                for di in range(CT):
                    nc.sync.dma_start(
                        out=kt[di].rearrange(
                            "p (k c) -> p k c",
                            c=128)[:, r + 8 * blk:r + 8 * blk + 7:2, :],
                        in_=kvr_d[blk][r, :,
                                      di * W:(di + 1) * W].rearrange(
                            "p (k c) -> p k c", c=128))
                for st4 in range(OST):
                    nc.sync.dma_start(
                        out=v_sb[r + 2 * (OST * blk + st4)][:],
                        in_=kvr_d[blk][r, :, HKT + st4 * (H * 65):
                                      HKT + (st4 + 1) * (H * 65)].rearrange(
                                          "p (h e) -> p h e", e=65))

        for cm in (psA_mm_cm, psA_st_cm, wA_cm, xn_cm, xt_cm, wqkv_cm):
            cm.__exit__(None, None, None)

        # ============ Phase B: attention + fused WO =====================
        woW_cm = tc.tile_pool(name="woW", bufs=1)
        woW = woW_cm.__enter__()
        wo_sb = []
        for ei in range(CT):
            wt = woW.tile([128, CT, 128], BF16, tag=f"wo{ei}",
                          name=f"wo{ei}")
            nc.sync.dma_start(out=wt[:], in_=wo_d[ei])
            wo_sb.append(wt)

        wB_cm = tc.tile_pool(name="workB", bufs=4)
        wB = wB_cm.__enter__()
        wR_cm = tc.tile_pool(name="rec", bufs=2)
        wR = wR_cm.__enter__()
        wC_cm = tc.tile_pool(name="workC", bufs=2)
        wC = wC_cm.__enter__()
        psB_sc_cm = tc.tile_pool(name="psB_sc", bufs=2, space="PSUM")
        psB_sc = psB_sc_cm.__enter__()
        psB_av_cm = tc.tile_pool(name="psB_av", bufs=2, space="PSUM")
        psB_av = psB_av_cm.__enter__()

        def norm_tail(st):
            """PE/DVE tail of softmax normalization; emitted during the
            NEXT (Bk, ht) block so its latency never stalls the PE."""
            t_Bk, t_ht, t_av, t_rec = st
            for hp in range(2):
                ps_bc = psB_sc.tile([128, 2 * W], F32, tag="ps_sc",
                                    name="bc")
                nc.tensor.matmul(
                    ps_bc[0:64, 0:W], onesm[0:1, 0:64],
                    t_rec[0:1, hp * W:(hp + 1) * W],
                    start=True, stop=True)
                bc_sb = wR.tile([64, W], F32, tag=f"bc_sb{hp}",
                                name=f"bc_sb{hp}")
                nc.vector.tensor_copy(out=bc_sb[:], in_=ps_bc[0:64, 0:W])
                if hp == 0:
                    nc.vector.tensor_mul(
                        av_sb[t_ht][0:64, t_Bk * W:(t_Bk + 1) * W],
                        t_av[hp][0:64, :], bc_sb[:])
                else:
                    avh1 = wR.tile([64, W], BF16, tag="avh1", name="avh1")
                    nc.vector.tensor_mul(avh1[:], t_av[hp][0:64, :],
                                         bc_sb[:])
                    nc.sync.dma_start(
                        out=av_sb[t_ht][64:128, t_Bk * W:(t_Bk + 1) * W],
                        in_=avh1[:])

        def wo_block(Bk, ei, xres):
            """One WO output tile (fused into the attention stream)."""
            ps = psB_sc.tile([128, W], F32, tag="ps_sc", name="ps_wo")
            for ci in range(CT):
                nc.tensor.matmul(ps[:], wo_sb[ei][:, ci, :],
                                 av_sb[ci][:, Bk * W:(Bk + 1) * W],
                                 start=(ci == 0), stop=(ci == CT - 1))
            osb = wC.tile([128, W], F32R, tag="osb")
            nc.vector.scalar_tensor_tensor(
                out=osb[:], in0=ps[:],
                scalar=boeff_sb[:, ei:ei + 1], in1=xres[:],
                op0=OP.add, op1=OP.add)
            nc.sync.dma_start(
                out=x2_d[ei * 128:(ei + 1) * 128, Bk * W:(Bk + 1) * W],
                in_=osb[:])

        def wo_res(Bk, ei):
            xres = wC.tile([128, W], F32R, tag="xres")
            nc.sync.dma_start(
                out=xres[:],
                in_=xto_d[:, ei, Bk * W:(Bk + 1) * W].bitcast(F32R))
            return xres

        def av_pair(ctx, pi, pex, plo):
            c_ht, c_ns = ctx[0], ctx[1]
            if ctx[2] is None:
                # lazy PSUM alloc: by now the norm_tail of the block two
                # back has been emitted, so the bufs=2 ring is safe
                ctx[2] = [psB_av.tile([128, W], F32, tag=f"ps_av{hp}",
                                      name=f"ps_av{hp}")
                          for hp in range(2)]
            for hp in range(2):
                nc.tensor.matmul(ctx[2][hp][0:65, plo:W],
                                 v_sb[pi][:, 2 * c_ht + hp, :],
                                 pex[:, hp * W + plo:(hp + 1) * W],
                                 start=(pi == 0),
                                 stop=(pi == c_ns - 1))

        def finish_block(fb):
            """Emit the sumexp reciprocal for a fully-AV'd block; returns
            the norm_tail pending record."""
            f_Bk, f_ht, f_ctx = fb
            se = wR.tile([128, 2 * W], F32, tag="se")
            for hp in range(2):
                nc.vector.tensor_copy(out=se[64:65, hp * W:(hp + 1) * W],
                                      in_=f_ctx[2][hp][64:65, :])
            # reciprocal_approx_fast only works at base partition 0 on HW;
            # DMA-shift the sumexp row down first
            se0 = wR.tile([1, 2 * W], F32, tag="se0")
            nc.sync.dma_start(out=se0[0:1, :], in_=se[64:65, :])
            recf = wR.tile([1, 2 * W], F32, tag="recf")
            nc.vector.reciprocal_approx_fast(out=recf[0:1, :],
                                             in_=se0[0:1, :])
            rec = wR.tile([1, 2 * W], F32R, tag="rec")
            nc.vector.tensor_copy(out=rec[:], in_=recf[:])
            return (f_Bk, f_ht, f_ctx[2], rec)

        def retire(pend):
            """norm_tail + fused WO for the block one behind `fin`."""
            norm_tail(pend)
            t_Bk, t_ht = pend[0], pend[1]
            if t_Bk > 0:
                wo_block(t_Bk - 1, t_ht, wo_res(t_Bk - 1, t_ht))

        # flat cross-block stream: scores/exp/mask of the NEXT block start
        # while the tail AVs / sumexp / norm_tail of the previous one are
        # still in flight, so the PE never drains at block boundaries.
        avq = []        # deferred AV pairs, two behind the score stream
        fin = None      # block whose AVs are tailing through avq
        pending = None  # block awaiting norm_tail
        for Bk in range(NB):
            for ht in range(HT):
                n_s = 8 * Bk + 8
                ctx = [ht, n_s, None]
                for i in range(n_s):
                    lo = _ja(i, Bk) * 128
                    ps_sc = psB_sc.tile([128, 2 * W], F32, tag="ps_sc",
                                        name="ps_sc")
                    for hp in range(2):
                        nc.tensor.matmul(
                            ps_sc[:, hp * W + lo:(hp + 1) * W],
                            kt[ht][64 * hp:64 * hp + 64,
                                   i * 128:(i + 1) * 128],
                            qt[ht][64 * hp:64 * hp + 64,
                                   Bk * W + lo:(Bk + 1) * W],
                            start=True, stop=True)
                    ex = wB.tile([128, 2 * W], BF16, tag="exp", name="ex")
                    nc.scalar.activation(
                        out=ex[:].rearrange("p (h w) -> p h w",
                                            h=2)[:, :, lo:W],
                        in_=ps_sc[:].rearrange("p (h w) -> p h w",
                                               h=2)[:, :, lo:W],
                        func=AF.Exp, scale=scale)
                    m = i - 8 * Bk
                    if m >= 0:
                        m2 = m % 2
                        for hp in range(2):
                            nc.vector.tensor_mul(
                                ex[:, hp * W + lo:hp * W + lo + 128],
                                ex[:, hp * W + lo:hp * W + lo + 128],
                                masks_sb[:, m2, :])
                    if len(avq) == 2:
                        av_pair(*avq.pop(0))
                    avq.append([ctx, i, ex, lo])
                    # previous block's AVs all emitted -> its reciprocal;
                    # the block before that retires (norm_tail + WO)
                    if fin is not None and avq[0][0] is ctx:
                        nxt = finish_block(fin)
                        fin = None
                        if pending is not None:
                            retire(pending)
                        pending = nxt
                fin = (Bk, ht, ctx)
        # drain the tail
        for pr in avq:
            av_pair(*pr)
        if pending is not None:
            retire(pending)
        norm_tail(finish_block(fin))
        # fused-WO tiles not covered by the ht stream
        wo_block(NB - 2, HT - 1, wo_res(NB - 2, HT - 1))
        for ei in range(CT):
            wo_block(NB - 1, ei, wo_res(NB - 1, ei))

        for cm in (psB_av_cm, psB_sc_cm, wC_cm, wR_cm, wB_cm, woW_cm):
            cm.__exit__(None, None, None)
        av_cm.__exit__(None, None, None)
        attio_cm.__exit__(None, None, None)

        # ============ Phase D: LN2 (transposed layout) ==================
        wD_cm = tc.tile_pool(name="workD", bufs=2)
        wD = wD_cm.__enter__()
        x2r_cm = tc.tile_pool(name="x2rp", bufs=1)
        p_x2r = x2r_cm.__enter__()
        psD_st_cm = tc.tile_pool(name="psD_st", bufs=2, space="PSUM")
        psD_st = psD_st_cm.__enter__()
        psD_bc_cm = tc.tile_pool(name="psD_bc", bufs=2, space="PSUM")
        psD_bc = psD_bc_cm.__enter__()

        for nb in range(NB):
            x2r = [p_x2r.tile([128, W], F32R, tag=f"x2r{ci}",
                              name=f"x2r{ci}") for ci in range(CT)]
            for ci in range(CT):
                nc.sync.dma_start(
                    out=x2r[ci][:],
                    in_=x2_d[ci * 128:(ci + 1) * 128, nb * W:(nb + 1) * W])
            ps_mu = psD_st.tile([1, W], F32, tag="ps_mu")
            ps_sq = psD_st.tile([1, W], F32, tag="ps_sq")
            for ci in range(CT):
                nc.tensor.matmul(ps_mu[:], ones_c[:], x2r[ci][:],
                                 start=(ci == 0), stop=(ci == CT - 1))
                sq = wD.tile([128, W], F32R, tag="sq")
                nc.scalar.activation(out=sq[:], in_=x2r[ci][:],
                                     func=AF.Square)
                nc.tensor.matmul(ps_sq[:], ones_c[:], sq[:],
                                 start=(ci == 0), stop=(ci == CT - 1))
            mur = wD.tile([1, W], F32R, tag="mur", bufs=1)
            nc.vector.tensor_scalar_mul(out=mur[:], in0=ps_mu[:],
                                        scalar1=1.0 / D)
            mu2 = wD.tile([1, W], F32, tag="mu2", bufs=1)
            nc.vector.tensor_mul(mu2[:], mur[:], mur[:])
            varr = wD.tile([1, W], F32, tag="varr", bufs=1)
            nc.vector.tensor_scalar_mul(out=varr[:], in0=ps_sq[:],
                                        scalar1=1.0 / D)
            nc.vector.tensor_sub(varr[:], varr[:], mu2[:])
            stdr = wD.tile([1, W], F32, tag="stdr", bufs=1)
            nc.scalar.activation(out=stdr[:], in_=varr[:], func=AF.Sqrt,
                                 bias=eps_c[0:1, :])
            rstdf = wD.tile([1, W], F32, tag="rstdf", bufs=1)
            nc.vector.reciprocal_approx_fast(out=rstdf[:], in_=stdr[:])
            rstdr = wD.tile([1, W], F32R, tag="rstdr", bufs=1)
            nc.vector.tensor_copy(out=rstdr[:], in_=rstdf[:])
            ps_mub = psD_bc.tile([128, W], F32, tag="ps_mub")
            nc.tensor.matmul(ps_mub[:], ones_r[:], mur[:],
                             start=True, stop=True)
            ps_rsb = psD_bc.tile([128, W], F32, tag="ps_rsb")
            nc.tensor.matmul(ps_rsb[:], ones_r[:], rstdr[:],
                             start=True, stop=True)
            for ci in range(CT):
                t1 = wD.tile([128, W], F32, tag="t1")
                nc.vector.tensor_sub(t1[:], x2r[ci][:], ps_mub[:])
                nc.vector.tensor_mul(x2nt[ci][:, nb * W:(nb + 1) * W],
                                     t1[:], ps_rsb[:])

        for cm in (psD_bc_cm, psD_st_cm, x2r_cm, wD_cm):
            cm.__exit__(None, None, None)

        # ============ Phase E: MLP ======================================
        ht_cm = tc.tile_pool(name="hpool", bufs=1)
        p_ht = ht_cm.__enter__()
        h_sb = [p_ht.tile([128, OWN_L], BF16, tag=f"h{i}", name=f"h{i}")
                for i in range(FT)]
        wE_cm = tc.tile_pool(name="workE", bufs=2)
        wE = wE_cm.__enter__()
        psE_cm = tc.tile_pool(name="psE", bufs=4, space="PSUM")
        psE = psE_cm.__enter__()

        for f in range(FT):
            wtile = wE.tile([128, CT, 128], BF16, tag="w1_lhsT")
            nc.sync.dma_start(out=wtile[:], in_=w1_d[f])
            for nb in range(NB):
                ps = psE.tile([128, W], F32, tag="ps_h")
                for ci in range(CT):
                    nc.tensor.matmul(ps[:], wtile[:, ci, :],
                                     x2nt[ci][:, nb * W:(nb + 1) * W],
                                     start=(ci == 0), stop=(ci == CT - 1))
                nc.scalar.activation(out=h_sb[f][:, nb * W:(nb + 1) * W],
                                     in_=ps[:], func=AF.Relu,
                                     bias=b1_sb[:, f:f + 1])
        for ei in range(CT):
            wtile = wE.tile([128, FT, 128], BF16, tag="w2_lhsT")
            nc.sync.dma_start(out=wtile[:], in_=w2_d[ei])
            for nb in range(NB):
                x2res = wE.tile([128, W], F32R, tag="x2res")
                nc.sync.dma_start(
                    out=x2res[:],
                    in_=x2_d[ei * 128:(ei + 1) * 128, nb * W:(nb + 1) * W])
                ps = psE.tile([128, W], F32, tag="ps_o2")
                for f in range(FT):
                    nc.tensor.matmul(ps[:], wtile[:, f, :],
                                     h_sb[f][:, nb * W:(nb + 1) * W],
                                     start=(f == 0), stop=(f == FT - 1))
                osb = wE.tile([128, W], F32, tag="osb")
                nc.vector.scalar_tensor_tensor(
                    out=osb[:], in0=ps[:], scalar=b2_sb[:, ei:ei + 1],
                    in1=x2res[:],
                    op0=OP.add, op1=OP.add)
                nc.sync.dma_start(
                    out=out_d[ei * 128:(ei + 1) * 128, nb * W:(nb + 1) * W],
                    in_=osb[:])

        for cm in (psE_cm, wE_cm, ht_cm, x2nt_cm, consts_cm):
            cm.__exit__(None, None, None)

    nc.compile()
    return nc, g


def make_masks(p):
    """Boundary masks [128, 2, 128] (bf16 0/1) for key tile i in query
    block Bk at own chunk (i-8Bk)//2, indexed by m2 = (i-8Bk) % 2.
    Own tiles interleave at 128-row granularity: own tile k is global
    tile p+2k, so the diagonal (i == p+2k) gets the universal triangle
    mask; off-parity offsets are all-ones (p=1) or all-zeros (p=0)."""
    kk = np.arange(128)[:, None]
    cc = np.arange(128)[None, :]
    tri = (cc >= kk).astype(np.float32)
    out = np.zeros((128, 2, 128), np.float32)
    if p == 0:
        out[:, 0, :] = tri
        out[:, 1, :] = 0.0
    else:
        out[:, 0, :] = 1.0
        out[:, 1, :] = tri
    return out.astype(BF)


def _tile_lhsT(wmat):
    """[K, M] -> [m, p, c, q] with out[m, p, c, q] = wmat[128c+p, 128m+q]."""
    K, M = wmat.shape
    CTl, MT = K // 128, M // 128
    w = wmat.reshape(CTl, 128, MT, 128)
    return np.ascontiguousarray(w.transpose(2, 1, 0, 3))


def prep_in_maps(inputs, L=L_, D=D_, H=H_, DFF=DFF_, Bn=B_):
    f64 = lambda k: np.asarray(inputs[k], np.float64)
    X = np.asarray(inputs["X"], np.float32)
    WQ, WK, WV, WO = f64("WQ"), f64("WK"), f64("WV"), f64("WO")
    W1, W2 = f64("W1"), f64("W2")
    bQ, bK, bV, bO = f64("bQ"), f64("bK"), f64("bV"), f64("bO")
    b1, b2 = f64("b1"), f64("b2")
    g1, be1, g2, be2 = f64("g1"), f64("be1"), f64("g2"), f64("be2")

    g = _derived(L, D, H, DFF)
    CT, FT = g["CT"], g["FT"]
    OWN_L, n_lt = g["OWN_L"], g["n_lt"]

    # fold LayerNorm affine transforms into the downstream weights
    WQf, bQf = g1[:, None] * WQ, bQ + be1 @ WQ
    WKf, bKf = g1[:, None] * WK, bK + be1 @ WK
    WVf, bVf = g1[:, None] * WV, bV + be1 @ WV
    boeff = bO + WO.T @ bVf
    W1f, b1f = g2[:, None] * W1, b1 + be2 @ W1

    c32 = lambda a: np.ascontiguousarray(a).astype(np.float32)
    wq_t = np.ascontiguousarray(
        _tile_lhsT(WQf).transpose(1, 0, 2, 3)).astype(BF)
    wk_t = np.ascontiguousarray(
        _tile_lhsT(WKf).transpose(1, 0, 2, 3)).astype(BF)
    wv_r = np.ascontiguousarray(
        WVf.reshape(CT, 128, D).transpose(1, 0, 2)).astype(BF)
    wo_t = _tile_lhsT(WO).astype(BF)
    w1_t = _tile_lhsT(W1f).astype(BF)
    w2_t = _tile_lhsT(W2).astype(BF)

    def cols(v, nt):
        return c32(np.reshape(v, (nt, 128)).T)

    common = dict(
        wq=wq_t, wk=wk_t, wv=wv_r, wo=wo_t, w1=w1_t, w2=w2_t,
        bqc=cols(bQf, CT), bkc=cols(bKf, CT), b1c=cols(b1f, FT),
        boeffc=cols(boeff, CT), b2c=cols(b2, CT),
        onescv=np.ones((128, 1), np.float32),
        onesrv=np.ones((1, 128), np.float32),
    )
    masks_by_p = [make_masks(p) for p in range(2)]

    in_maps = []
    for core in range(2 * Bn):
        b, p = core // 2, core % 2
        m = dict(common)
        # own rows: 128-row tiles p, p+2, ... of X[b]
        xo = X[b].reshape(n_lt, 128, D)[p::2].reshape(OWN_L, D)
        m["xto"] = np.ascontiguousarray(
            xo.T.reshape(CT, 128, OWN_L).transpose(1, 0, 2))
        m["masks"] = masks_by_p[p]
        in_maps.append(m)
    return in_maps


def gather(results, L=L_, D=D_, Bn=B_):
    n_own = (L // 128) // 2
    out = np.empty((Bn, L, D), np.float32)
    for core, r in enumerate(results):
        b, p = core // 2, core % 2
        part = np.ascontiguousarray(r["outT"].T)
        for k in range(n_own):
            out[b, 128 * (p + 2 * k):128 * (p + 2 * k) + 128, :] = \
                part[128 * k:128 * (k + 1), :]
    return out


_NC_CACHE = {}


def get_nc():
    if "nc" not in _NC_CACHE:
        _NC_CACHE["nc"] = build_nc()
    return _NC_CACHE["nc"]


def kernel(**inputs) -> np.ndarray:
    nc, _ = get_nc()
    in_maps = prep_in_maps(inputs)
    res = run_bass_kernel_spmd(nc, in_maps, list(range(N_CORES)))
    return gather(res.results)



# revision 6
# speedup vs baseline: 1.2459x; 1.2459x over previous
"""Trainium2 Bass kernel for a dense transformer block (causal attn + MLP).

Problem: B=4, L=2048, D=1024, H=16 (DH=64), DFF=4096, fp32 in/out.

Sharding: 8 cores = 4 batches x 2 parity groups. Core c handles batch
b=c//2 and query-row tiles {p, p+2, ..., p+14} (p=c%2); interleaved
128-row tiles balance causal-attention work between the two cores of a
batch.

v3 design (vs v2):
- K/V are computed for the FULL sequence on every core (replicated)
  instead of being exchanged through an HBM AllGather; this removes the
  collective waits and the DMA congestion that starved the Q projection
  in v2 at the cost of ~55us of extra (well-overlapped) PE work.
- The host permutes each 1024-row window of X so the core's own-parity
  128-row tiles come first. Own queries are then the first 512 columns
  of every window (contiguous), keys stay tile-granular causal with the
  same two 128-wide masks (triangle / parity ones-or-zeros), and kt /
  v_sb are indexed by permuted (physical) tile id throughout.
- LN1 runs per 512-column block in the transposed layout, software-
  pipelined: stats of block g+1 are interleaved between the K and V
  matmuls of block g so the PE never waits on the DVE stat chain.
- Attention, deferred softmax normalization through a ones-column in V,
  fused WO, DRAM-resident x2, LN2 and the MLP are as in v2.
"""

import numpy as np
import ml_dtypes

import concourse.bacc as bacc
import concourse.bass as bass
import concourse.mybir as mybir
import concourse.tile as tile
from concourse.bass_utils import run_bass_kernel_spmd

F32 = mybir.dt.float32
F32R = mybir.dt.float32r
BF16 = mybir.dt.bfloat16
BF = ml_dtypes.bfloat16
EPS = 1e-5
AF = mybir.ActivationFunctionType
OP = mybir.AluOpType

B_, L_, D_, H_, DFF_ = 4, 2048, 1024, 16, 4096
N_CORES = 8


def _derived(L, D, H, DFF):
    CT = D // 128
    FT = DFF // 128
    n_lt = L // 128
    n_own = n_lt // 2
    OWN_L = n_own * 128
    NB = OWN_L // 512
    HT = H // 2
    assert CT == HT
    return dict(CT=CT, FT=FT, n_lt=n_lt, n_own=n_own, OWN_L=OWN_L, NB=NB,
                HT=HT)


def build_nc(L=L_, D=D_, H=H_, DFF=DFF_, n_cores=N_CORES):
    g = _derived(L, D, H, DFF)
    CT, FT = g["CT"], g["FT"]
    n_lt, n_own, OWN_L, NB, HT = (g["n_lt"], g["n_own"], g["OWN_L"],
                                  g["NB"], g["HT"])
    W = 512
    NG = L // W                # global 512-col blocks (4)
    TPG = W // 128             # 128-tiles per block (4)
    scale = 1.0 / 8.0          # 1/sqrt(DH)

    nc = bacc.Bacc("TRN2", target_bir_lowering=False, debug=False,
                   num_devices=n_cores)

    dp = nc.declare_dram_parameter
    xto_d = dp("xto", [128, CT, OWN_L], F32, isOutput=False)   # own rows f32
    xtob_d = dp("xtob", [128, CT, L], BF16, isOutput=False)    # all rows bf16
    wq_d = dp("wq", [128, CT, CT, 128], BF16, isOutput=False)  # [p,d,c,q]
    wk_d = dp("wk", [128, CT, CT, 128], BF16, isOutput=False)
    wv_d = dp("wv", [128, CT, D], BF16, isOutput=False)        # [p,c,dv]
    wo_d = dp("wo", [CT, 128, CT, 128], BF16, isOutput=False)  # [e,p,c,q]
    w1_d = dp("w1", [FT, 128, CT, 128], BF16, isOutput=False)  # [f,p,c,q]
    w2_d = dp("w2", [CT, 128, FT, 128], BF16, isOutput=False)  # [e,p,f,q]
    bq_d = dp("bqc", [128, CT], F32, isOutput=False)
    bk_d = dp("bkc", [128, CT], F32, isOutput=False)
    b1_d = dp("b1c", [128, FT], F32, isOutput=False)
    boeff_d = dp("boeffc", [128, CT], F32, isOutput=False)
    b2_d = dp("b2c", [128, CT], F32, isOutput=False)
    onescv_d = dp("onescv", [128, 1], F32, isOutput=False)
    onesrv_d = dp("onesrv", [1, 128], F32, isOutput=False)
    masks_d = dp("masks", [128, 2, 128], BF16, isOutput=False)
    out_d = dp("outT", [D, OWN_L], F32, isOutput=True)
    x2_d = nc.dram_tensor("x2_dram", [D, OWN_L], F32R)

    with tile.TileContext(nc) as tc, \
         nc.allow_low_precision(reason="bf16/f32r matmul operands by design"):
        consts_cm = tc.tile_pool(name="consts", bufs=1)
        consts = consts_cm.__enter__()

        eps_c = consts.tile([128, 1], F32, tag="eps")
        nc.vector.memset(eps_c[:], EPS)
        ones_c = consts.tile([128, 1], F32R, tag="onesc")
        nc.sync.dma_start(out=ones_c[:], in_=onescv_d[:].bitcast(F32R))
        ones_cb = consts.tile([128, 1], BF16, tag="onescb")
        nc.vector.memset(ones_cb[:], 1.0)
        ones_r = consts.tile([1, 128], F32R, tag="onesr")
        nc.sync.dma_start(out=ones_r[:], in_=onesrv_d[:].bitcast(F32R))
        _oap = onesrv_d[:]
        onesm = consts.tile([128, 128], F32R, tag="onesm")
        nc.sync.dma_start(out=onesm[:], in_=bass.AP(
            tensor=_oap.tensor, offset=_oap.offset,
            ap=[[0, 128], [1, 128]]).bitcast(F32R))
        bq_sb = consts.tile([128, CT], F32, tag="bq")
        nc.sync.dma_start(out=bq_sb[:], in_=bq_d[:])
        bk_sb = consts.tile([128, CT], F32, tag="bk")
        nc.sync.dma_start(out=bk_sb[:], in_=bk_d[:])
        b1_sb = consts.tile([128, FT], F32, tag="b1")
        nc.sync.dma_start(out=b1_sb[:], in_=b1_d[:])
        boeff_sb = consts.tile([128, CT], F32, tag="boeff")
        nc.sync.dma_start(out=boeff_sb[:], in_=boeff_d[:])
        b2_sb = consts.tile([128, CT], F32, tag="b2")
        nc.sync.dma_start(out=b2_sb[:], in_=b2_d[:])
        masks_sb = consts.tile([128, 2, 128], BF16, tag="masks")
        nc.sync.dma_start(out=masks_sb[:], in_=masks_d[:])

        # long-lived SBUF state (open before phase-local pools; LIFO)
        x2nt_cm = tc.tile_pool(name="x2nt", bufs=1)
        p_x2nt = x2nt_cm.__enter__()
        x2nt = [p_x2nt.tile([128, OWN_L], BF16, tag=f"x2nt{i}",
                            name=f"x2nt{i}") for i in range(CT)]
        attio_cm = tc.tile_pool(name="attio", bufs=1)
        attio = attio_cm.__enter__()
        kt = [attio.tile([128, L], BF16, tag=f"kt{i}", name=f"kt{i}")
              for i in range(CT)]
        qt = [attio.tile([128, OWN_L], BF16, tag=f"qt{i}", name=f"qt{i}")
              for i in range(CT)]
        v_sb = [attio.tile([128, H, 65], BF16, tag=f"v{i}", name=f"v{i}")
                for i in range(n_lt)]
        av_cm = tc.tile_pool(name="avp", bufs=1)
        p_av = av_cm.__enter__()
        av_sb = [p_av.tile([128, OWN_L], BF16, tag=f"av{i}", name=f"av{i}")
                 for i in range(CT)]

        # ============ Phase A: LN1 (transposed) + full-L K/V + Q ========
        wqkv_cm = tc.tile_pool(name="wqkv", bufs=1)
        wqkv = wqkv_cm.__enter__()
        xt_cm = tc.tile_pool(name="xtp", bufs=1)
        p_xt = xt_cm.__enter__()
        xn_cm = tc.tile_pool(name="xnp", bufs=1)
        p_xn = xn_cm.__enter__()
        wA_cm = tc.tile_pool(name="workA", bufs=2)
        wA = wA_cm.__enter__()
        psA_st_cm = tc.tile_pool(name="psA_st", bufs=2, space="PSUM")
        psA_st = psA_st_cm.__enter__()
        psA_mm_cm = tc.tile_pool(name="psA_mm", bufs=4, space="PSUM")
        psA_mm = psA_mm_cm.__enter__()

        wv_sb = wqkv.tile([128, CT, D], BF16, tag="wv", name="wv_sb")
        nc.sync.dma_start(out=wv_sb[:], in_=wv_d[:])
        wk_sb = wqkv.tile([128, CT, CT, 128], BF16, tag="wk", name="wk_sb")
        nc.sync.dma_start(out=wk_sb[:], in_=wk_d[:])

        # ones column of every V tile (sumexp accumulator), set once; the
        # per-head 0:64 chunks are overwritten by the V projection copies
        for i in range(n_lt):
            nc.vector.memset(v_sb[i][:], 1.0)

        # xn ping-pong: slot g%2 holds LN1 output of 512-col block g
        xn_t = p_xn.tile([128, CT, 2, W], BF16, tag="xn", name="xn_t")

        xts = [None] * NG

        def ln_front(gb):
            """xt DMA + ACT squares + PE stat matmuls for block gb."""
            xt = [p_xt.tile([128, W], BF16, tag=f"xt{ci}", bufs=1,
                            name=f"xt{ci}") for ci in range(CT)]
            for ci in range(CT):
                nc.sync.dma_start(
                    out=xt[ci][:], in_=xtob_d[:, ci, gb * W:(gb + 1) * W])
            xts[gb] = xt
            sqs = []
            for ci in range(CT):
                sq = wA.tile([128, W], BF16, tag="sq", bufs=CT)
                nc.scalar.activation(out=sq[:], in_=xt[ci][:],
                                     func=AF.Square)
                sqs.append(sq)
            ps_mu = psA_st.tile([1, W], F32, tag="ps_mu")
            ps_sq = psA_st.tile([1, W], F32, tag="ps_sq")
            for ci in range(CT):
                nc.tensor.matmul(ps_mu[:], ones_cb[:], xt[ci][:],
                                 start=(ci == 0), stop=(ci == CT - 1))
            for ci in range(CT):
                nc.tensor.matmul(ps_sq[:], ones_cb[:], sqs[ci][:],
                                 start=(ci == 0), stop=(ci == CT - 1))
            return ps_mu, ps_sq

        def ln_back(gb, st):
            """DVE stat chain + PE broadcast + DVE normalize for gb."""
            ps_mu, ps_sq = st
            mur = wA.tile([1, W], F32R, tag="mur", bufs=1)
            nc.vector.tensor_scalar_mul(out=mur[:], in0=ps_mu[:],
                                        scalar1=1.0 / D)
            mu2 = wA.tile([1, W], F32, tag="mu2", bufs=1)
            nc.vector.tensor_mul(mu2[:], mur[:], mur[:])
            varr = wA.tile([1, W], F32, tag="varr", bufs=1)
            nc.vector.tensor_scalar_mul(out=varr[:], in0=ps_sq[:],
                                        scalar1=1.0 / D)
            nc.vector.tensor_sub(varr[:], varr[:], mu2[:])
            stdr = wA.tile([1, W], F32, tag="stdr", bufs=1)
            nc.scalar.activation(out=stdr[:], in_=varr[:], func=AF.Sqrt,
                                 bias=eps_c[0:1, :])
            rstdf = wA.tile([1, W], F32, tag="rstdf", bufs=1)
            nc.vector.reciprocal_approx_fast(out=rstdf[:], in_=stdr[:])
            rstdr = wA.tile([1, W], F32R, tag="rstdr", bufs=1)
            nc.vector.tensor_copy(out=rstdr[:], in_=rstdf[:])
            ps_mub = psA_mm.tile([128, W], F32, tag="ps_mm", name="ps_mub")
            nc.tensor.matmul(ps_mub[:], ones_r[:], mur[:],
                             start=True, stop=True)
            ps_rsb = psA_mm.tile([128, W], F32, tag="ps_mm", name="ps_rsb")
            nc.tensor.matmul(ps_rsb[:], ones_r[:], rstdr[:],
                             start=True, stop=True)
            sl = gb % 2
            for ci in range(CT):
                t1 = wA.tile([128, W], F32, tag="t1")
                nc.vector.tensor_sub(t1[:], xts[gb][ci][:], ps_mub[:])
                nc.vector.tensor_mul(xn_t[:, ci, sl, :], t1[:], ps_rsb[:])

        def k_block(gb):
            sl = gb % 2
            for di in range(CT):
                ps = psA_mm.tile([128, W], F32, tag="ps_mm", name="ps_k")
                for ci in range(CT):
                    nc.tensor.matmul(ps[:], wk_sb[:, di, ci, :],
                                     xn_t[:, ci, sl, :],
                                     start=(ci == 0), stop=(ci == CT - 1))
                nc.vector.tensor_scalar_add(
                    out=kt[di][:, gb * W:(gb + 1) * W], in0=ps[:],
                    scalar1=bk_sb[:, di:di + 1])

        def v_block(gb, half):
            sl = gb % 2
            nhh = W // 64
            for st4 in (range(0, TPG // 2) if half == 0
                        else range(TPG // 2, TPG)):
                ti = TPG * gb + st4
                for vb in range(D // W):
                    ps = psA_mm.tile([128, W], F32, tag="ps_mm", name="ps_v")
                    for ci in range(CT):
                        nc.tensor.matmul(
                            ps[:],
                            xn_t[:, ci, sl, st4 * 128:(st4 + 1) * 128],
                            wv_sb[:, ci, vb * W:(vb + 1) * W],
                            start=(ci == 0), stop=(ci == CT - 1))
                    nc.vector.tensor_copy(
                        out=v_sb[ti][:, vb * nhh:(vb + 1) * nhh, 0:64],
                        in_=ps[:].rearrange("p (h d) -> p h d", d=64))

        def q_block(ob):
            """Q for own 512-block ob (= first half of window ob)."""
            sl = (2 * ob) % 2  # == 0; own half lives in slot 0
            for di in range(CT):
                wqt = wqkv.tile([128, CT, 128], BF16, tag="wq",
                                name="wqt", bufs=2)
                nc.sync.dma_start(out=wqt[:], in_=wq_d[:, di])
                ps = psA_mm.tile([128, W], F32, tag="ps_mm", name="ps_q")
                for ci in range(CT):
                    nc.tensor.matmul(ps[:], wqt[:, ci, :],
                                     xn_t[:, ci, sl, :],
                                     start=(ci == 0), stop=(ci == CT - 1))
                nc.vector.tensor_scalar_add(
                    out=qt[di][:, ob * W:(ob + 1) * W], in0=ps[:],
                    scalar1=bq_sb[:, di:di + 1])

        st = ln_front(0)
        ln_back(0, st)
        for gb in range(NG):
            k_block(gb)
            st = ln_front(gb + 1) if gb + 1 < NG else None
            v_block(gb, 0)
            if st is not None:
                ln_back(gb + 1, st)
            v_block(gb, 1)
            if gb % 2 == 0:
                q_block(gb // 2)

        for cm in (psA_mm_cm, psA_st_cm, wA_cm, xn_cm, xt_cm, wqkv_cm):
            cm.__exit__(None, None, None)

        # ============ Phase B: attention + fused WO =====================
        # Physical (permuted) key-tile order: window w holds global tiles
        # 8w..8w+7 as [own j=0..3 | other j=0..3]; own query block Bk ==
        # window Bk. Keys in windows < Bk are fully valid; in window Bk,
        # phys tile j (own or other) starts at query chunk j%4: own gets
        # the universal triangle there, other gets the parity mask
        # (ones for p=1, zeros for p=0).
        woW_cm = tc.tile_pool(name="woW", bufs=1)
        woW = woW_cm.__enter__()
        wo_sb = []
        for ei in range(CT):
            wt = woW.tile([128, CT, 128], BF16, tag=f"wo{ei}",
                          name=f"wo{ei}")
            nc.sync.dma_start(out=wt[:], in_=wo_d[ei])
            wo_sb.append(wt)

        wB_cm = tc.tile_pool(name="workB", bufs=4)
        wB = wB_cm.__enter__()
        wR_cm = tc.tile_pool(name="rec", bufs=2)
        wR = wR_cm.__enter__()
        wC_cm = tc.tile_pool(name="workC", bufs=2)
        wC = wC_cm.__enter__()
        psB_sc_cm = tc.tile_pool(name="psB_sc", bufs=2, space="PSUM")
        psB_sc = psB_sc_cm.__enter__()
        psB_av_cm = tc.tile_pool(name="psB_av", bufs=2, space="PSUM")
        psB_av = psB_av_cm.__enter__()

        def norm_tail(st):
            """PE/DVE tail of softmax normalization; emitted during the
            NEXT (Bk, ht) block so its latency never stalls the PE."""
            t_Bk, t_ht, t_av, t_rec = st
            for hp in range(2):
                ps_bc = psB_sc.tile([128, 2 * W], F32, tag="ps_sc",
                                    name="bc")
                nc.tensor.matmul(
                    ps_bc[0:64, 0:W], onesm[0:1, 0:64],
                    t_rec[0:1, hp * W:(hp + 1) * W],
                    start=True, stop=True)
                bc_sb = wR.tile([64, W], F32, tag=f"bc_sb{hp}",
                                name=f"bc_sb{hp}")
                nc.vector.tensor_copy(out=bc_sb[:], in_=ps_bc[0:64, 0:W])
                if hp == 0:
                    nc.vector.tensor_mul(
                        av_sb[t_ht][0:64, t_Bk * W:(t_Bk + 1) * W],
                        t_av[hp][0:64, :], bc_sb[:])
                else:
                    avh1 = wR.tile([64, W], BF16, tag="avh1", name="avh1")
                    nc.vector.tensor_mul(avh1[:], t_av[hp][0:64, :],
                                         bc_sb[:])
                    nc.sync.dma_start(
                        out=av_sb[t_ht][64:128, t_Bk * W:(t_Bk + 1) * W],
                        in_=avh1[:])

        def wo_block(Bk, ei, xres):
            """One WO output tile (fused into the attention stream)."""
            ps = psB_sc.tile([128, W], F32, tag="ps_sc", name="ps_wo")
            for ci in range(CT):
                nc.tensor.matmul(ps[:], wo_sb[ei][:, ci, :],
                                 av_sb[ci][:, Bk * W:(Bk + 1) * W],
                                 start=(ci == 0), stop=(ci == CT - 1))
            osb = wC.tile([128, W], F32R, tag="osb")
            nc.vector.scalar_tensor_tensor(
                out=osb[:], in0=ps[:],
                scalar=boeff_sb[:, ei:ei + 1], in1=xres[:],
                op0=OP.add, op1=OP.add)
            nc.sync.dma_start(
                out=x2_d[ei * 128:(ei + 1) * 128, Bk * W:(Bk + 1) * W],
                in_=osb[:])

        def wo_res(Bk, ei):
            xres = wC.tile([128, W], F32R, tag="xres")
            nc.sync.dma_start(
                out=xres[:],
                in_=xto_d[:, ei, Bk * W:(Bk + 1) * W].bitcast(F32R))
            return xres

        def av_pair(ctx, pi, pex, plo):
            c_ht, c_ns = ctx[0], ctx[1]
            if ctx[2] is None:
                # lazy PSUM alloc: by now the norm_tail of the block two
                # back has been emitted, so the bufs=2 ring is safe
                ctx[2] = [psB_av.tile([128, W], F32, tag=f"ps_av{hp}",
                                      name=f"ps_av{hp}")
                          for hp in range(2)]
            for hp in range(2):
                nc.tensor.matmul(ctx[2][hp][0:65, plo:W],
                                 v_sb[pi][:, 2 * c_ht + hp, :],
                                 pex[:, hp * W + plo:(hp + 1) * W],
                                 start=(pi == 0),
                                 stop=(pi == c_ns - 1))

        def finish_block(fb):
            """Emit the sumexp reciprocal for a fully-AV'd block; returns
            the norm_tail pending record."""
            f_Bk, f_ht, f_ctx = fb
            se = wR.tile([128, 2 * W], F32, tag="se")
            for hp in range(2):
                nc.vector.tensor_copy(out=se[64:65, hp * W:(hp + 1) * W],
                                      in_=f_ctx[2][hp][64:65, :])
            # reciprocal_approx_fast only works at base partition 0 on HW;
            # DMA-shift the sumexp row down first
            se0 = wR.tile([1, 2 * W], F32, tag="se0")
            nc.sync.dma_start(out=se0[0:1, :], in_=se[64:65, :])
            recf = wR.tile([1, 2 * W], F32, tag="recf")
            nc.vector.reciprocal_approx_fast(out=recf[0:1, :],
                                             in_=se0[0:1, :])
            rec = wR.tile([1, 2 * W], F32R, tag="rec")
            nc.vector.tensor_copy(out=rec[:], in_=recf[:])
            return (f_Bk, f_ht, f_ctx[2], rec)

        def retire(pend):
            """norm_tail + fused WO for the block one behind `fin`."""
            norm_tail(pend)
            t_Bk, t_ht = pend[0], pend[1]
            if t_Bk > 0:
                wo_block(t_Bk - 1, t_ht, wo_res(t_Bk - 1, t_ht))

        # flat cross-block stream: scores/exp/mask of the NEXT block start
        # while the tail AVs / sumexp / norm_tail of the previous one are
        # still in flight, so the PE never drains at block boundaries.
        avq = []        # deferred AV pairs, two behind the score stream
        fin = None      # block whose AVs are tailing through avq
        pending = None  # block awaiting norm_tail
        for Bk in range(NB):
            for ht in range(HT):
                n_s = 8 * Bk + 8
                ctx = [ht, n_s, None]
                for i in range(n_s):
                    j = i - 8 * Bk
                    lo = 128 * (j % 4) if j >= 0 else 0
                    ps_sc = psB_sc.tile([128, 2 * W], F32, tag="ps_sc",
                                        name="ps_sc")
                    for hp in range(2):
                        nc.tensor.matmul(
                            ps_sc[:, hp * W + lo:(hp + 1) * W],
                            kt[ht][64 * hp:64 * hp + 64,
                                   i * 128:(i + 1) * 128],
                            qt[ht][64 * hp:64 * hp + 64,
                                   Bk * W + lo:(Bk + 1) * W],
                            start=True, stop=True)
                    ex = wB.tile([128, 2 * W], BF16, tag="exp", name="ex")
                    nc.scalar.activation(
                        out=ex[:].rearrange("p (h w) -> p h w",
                                            h=2)[:, :, lo:W],
                        in_=ps_sc[:].rearrange("p (h w) -> p h w",
                                               h=2)[:, :, lo:W],
                        func=AF.Exp, scale=scale)
                    if j >= 0:
                        mi = 0 if j < 4 else 1
                        for hp in range(2):
                            nc.vector.tensor_mul(
                                ex[:, hp * W + lo:hp * W + lo + 128],
                                ex[:, hp * W + lo:hp * W + lo + 128],
                                masks_sb[:, mi, :])
                    if len(avq) == 2:
                        av_pair(*avq.pop(0))
                    avq.append([ctx, i, ex, lo])
                    # previous block's AVs all emitted -> its reciprocal;
                    # the block before that retires (norm_tail + WO)
                    if fin is not None and avq[0][0] is ctx:
                        nxt = finish_block(fin)
                        fin = None
                        if pending is not None:
                            retire(pending)
                        pending = nxt
                fin = (Bk, ht, ctx)
        # drain the tail
        for pr in avq:
            av_pair(*pr)
        if pending is not None:
            retire(pending)
        norm_tail(finish_block(fin))
        # fused-WO tiles not covered by the ht stream
        wo_block(NB - 2, HT - 1, wo_res(NB - 2, HT - 1))
        for ei in range(CT):
            wo_block(NB - 1, ei, wo_res(NB - 1, ei))

        for cm in (psB_av_cm, psB_sc_cm, wC_cm, wR_cm, wB_cm, woW_cm):
            cm.__exit__(None, None, None)
        av_cm.__exit__(None, None, None)
        attio_cm.__exit__(None, None, None)

        # ============ Phase D: LN2 (transposed layout) ==================
        wD_cm = tc.tile_pool(name="workD", bufs=2)
        wD = wD_cm.__enter__()
        x2r_cm = tc.tile_pool(name="x2rp", bufs=1)
        p_x2r = x2r_cm.__enter__()
        psD_st_cm = tc.tile_pool(name="psD_st", bufs=2, space="PSUM")
        psD_st = psD_st_cm.__enter__()
        psD_bc_cm = tc.tile_pool(name="psD_bc", bufs=2, space="PSUM")
        psD_bc = psD_bc_cm.__enter__()

        for nb in range(NB):
            x2r = [p_x2r.tile([128, W], F32R, tag=f"x2r{ci}",
                              name=f"x2r{ci}") for ci in range(CT)]
            for ci in range(CT):
                nc.sync.dma_start(
                    out=x2r[ci][:],
                    in_=x2_d[ci * 128:(ci + 1) * 128, nb * W:(nb + 1) * W])
            ps_mu = psD_st.tile([1, W], F32, tag="ps_mu")
            ps_sq = psD_st.tile([1, W], F32, tag="ps_sq")
            for ci in range(CT):
                nc.tensor.matmul(ps_mu[:], ones_c[:], x2r[ci][:],
                                 start=(ci == 0), stop=(ci == CT - 1))
                sq = wD.tile([128, W], F32R, tag="sq")
                nc.scalar.activation(out=sq[:], in_=x2r[ci][:],
                                     func=AF.Square)
                nc.tensor.matmul(ps_sq[:], ones_c[:], sq[:],
                                 start=(ci == 0), stop=(ci == CT - 1))
            mur = wD.tile([1, W], F32R, tag="mur", bufs=1)
            nc.vector.tensor_scalar_mul(out=mur[:], in0=ps_mu[:],
                                        scalar1=1.0 / D)
            mu2 = wD.tile([1, W], F32, tag="mu2", bufs=1)
            nc.vector.tensor_mul(mu2[:], mur[:], mur[:])
            varr = wD.tile([1, W], F32, tag="varr", bufs=1)
            nc.vector.tensor_scalar_mul(out=varr[:], in0=ps_sq[:],
                                        scalar1=1.0 / D)
            nc.vector.tensor_sub(varr[:], varr[:], mu2[:])
            stdr = wD.tile([1, W], F32, tag="stdr", bufs=1)
            nc.scalar.activation(out=stdr[:], in_=varr[:], func=AF.Sqrt,
                                 bias=eps_c[0:1, :])
            rstdf = wD.tile([1, W], F32, tag="rstdf", bufs=1)
            nc.vector.reciprocal_approx_fast(out=rstdf[:], in_=stdr[:])
            rstdr = wD.tile([1, W], F32R, tag="rstdr", bufs=1)
            nc.vector.tensor_copy(out=rstdr[:], in_=rstdf[:])
            ps_mub = psD_bc.tile([128, W], F32, tag="ps_mub")
            nc.tensor.matmul(ps_mub[:], ones_r[:], mur[:],
                             start=True, stop=True)
            ps_rsb = psD_bc.tile([128, W], F32, tag="ps_rsb")
            nc.tensor.matmul(ps_rsb[:], ones_r[:], rstdr[:],
                             start=True, stop=True)
            for ci in range(CT):
                t1 = wD.tile([128, W], F32, tag="t1")
                nc.vector.tensor_sub(t1[:], x2r[ci][:], ps_mub[:])
                nc.vector.tensor_mul(x2nt[ci][:, nb * W:(nb + 1) * W],
                                     t1[:], ps_rsb[:])

        for cm in (psD_bc_cm, psD_st_cm, x2r_cm, wD_cm):
            cm.__exit__(None, None, None)

        # ============ Phase E: MLP ======================================
        ht_cm = tc.tile_pool(name="hpool", bufs=1)
        p_ht = ht_cm.__enter__()
        h_sb = [p_ht.tile([128, OWN_L], BF16, tag=f"h{i}", name=f"h{i}")
                for i in range(FT)]
        wE_cm = tc.tile_pool(name="workE", bufs=2)
        wE = wE_cm.__enter__()
        psE_cm = tc.tile_pool(name="psE", bufs=4, space="PSUM")
        psE = psE_cm.__enter__()

        for f in range(FT):
            wtile = wE.tile([128, CT, 128], BF16, tag="w1_lhsT")
            nc.sync.dma_start(out=wtile[:], in_=w1_d[f])
            for nb in range(NB):
                ps = psE.tile([128, W], F32, tag="ps_h")
                for ci in range(CT):
                    nc.tensor.matmul(ps[:], wtile[:, ci, :],
                                     x2nt[ci][:, nb * W:(nb + 1) * W],
                                     start=(ci == 0), stop=(ci == CT - 1))
                nc.scalar.activation(out=h_sb[f][:, nb * W:(nb + 1) * W],
                                     in_=ps[:], func=AF.Relu,
                                     bias=b1_sb[:, f:f + 1])
        for ei in range(CT):
            wtile = wE.tile([128, FT, 128], BF16, tag="w2_lhsT")
            nc.sync.dma_start(out=wtile[:], in_=w2_d[ei])
            for nb in range(NB):
                x2res = wE.tile([128, W], F32R, tag="x2res")
                nc.sync.dma_start(
                    out=x2res[:],
                    in_=x2_d[ei * 128:(ei + 1) * 128, nb * W:(nb + 1) * W])
                ps = psE.tile([128, W], F32, tag="ps_o2")
                for f in range(FT):
                    nc.tensor.matmul(ps[:], wtile[:, f, :],
                                     h_sb[f][:, nb * W:(nb + 1) * W],
                                     start=(f == 0), stop=(f == FT - 1))
                osb = wE.tile([128, W], F32, tag="osb")
                nc.vector.scalar_tensor_tensor(
                    out=osb[:], in0=ps[:], scalar=b2_sb[:, ei:ei + 1],
                    in1=x2res[:],
                    op0=OP.add, op1=OP.add)
                nc.sync.dma_start(
                    out=out_d[ei * 128:(ei + 1) * 128, nb * W:(nb + 1) * W],
                    in_=osb[:])

        for cm in (psE_cm, wE_cm, ht_cm, x2nt_cm, consts_cm):
            cm.__exit__(None, None, None)

    nc.compile()
    return nc, g


def make_masks(p):
    """[128, 2, 128] bf16 masks. Slot 0: universal triangle (applied on
    a core's OWN diagonal tiles). Slot 1: parity mask for the OTHER
    parity's diagonal tiles — all ones when the other parity's rows
    precede own rows (p=1), all zeros otherwise (p=0)."""
    kk = np.arange(128)[:, None]
    cc = np.arange(128)[None, :]
    tri = (cc >= kk).astype(np.float32)
    out = np.zeros((128, 2, 128), np.float32)
    out[:, 0, :] = tri
    out[:, 1, :] = 1.0 if p == 1 else 0.0
    return out.astype(BF)


def _tile_lhsT(wmat):
    """[K, M] -> [m, p, c, q] with out[m, p, c, q] = wmat[128c+p, 128m+q]."""
    K, M = wmat.shape
    CTl, MT = K // 128, M // 128
    w = wmat.reshape(CTl, 128, MT, 128)
    return np.ascontiguousarray(w.transpose(2, 1, 0, 3))


def prep_in_maps(inputs, L=L_, D=D_, H=H_, DFF=DFF_, Bn=B_):
    f64 = lambda k: np.asarray(inputs[k], np.float64)
    X = np.asarray(inputs["X"], np.float32)
    WQ, WK, WV, WO = f64("WQ"), f64("WK"), f64("WV"), f64("WO")
    W1, W2 = f64("W1"), f64("W2")
    bQ, bK, bV, bO = f64("bQ"), f64("bK"), f64("bV"), f64("bO")
    b1, b2 = f64("b1"), f64("b2")
    g1, be1, g2, be2 = f64("g1"), f64("be1"), f64("g2"), f64("be2")

    g = _derived(L, D, H, DFF)
    CT, FT = g["CT"], g["FT"]
    OWN_L, n_lt = g["OWN_L"], g["n_lt"]

    # fold LayerNorm affine transforms into the downstream weights
    WQf, bQf = g1[:, None] * WQ, bQ + be1 @ WQ
    WKf, bKf = g1[:, None] * WK, bK + be1 @ WK
    WVf, bVf = g1[:, None] * WV, bV + be1 @ WV
    boeff = bO + WO.T @ bVf
    W1f, b1f = g2[:, None] * W1, b1 + be2 @ W1

    c32 = lambda a: np.ascontiguousarray(a).astype(np.float32)
    wq_t = np.ascontiguousarray(
        _tile_lhsT(WQf).transpose(1, 0, 2, 3)).astype(BF)
    wk_t = np.ascontiguousarray(
        _tile_lhsT(WKf).transpose(1, 0, 2, 3)).astype(BF)
    wv_r = np.ascontiguousarray(
        WVf.reshape(CT, 128, D).transpose(1, 0, 2)).astype(BF)
    wo_t = _tile_lhsT(WO).astype(BF)
    w1_t = _tile_lhsT(W1f).astype(BF)
    w2_t = _tile_lhsT(W2).astype(BF)

    def cols(v, nt):
        return c32(np.reshape(v, (nt, 128)).T)

    common = dict(
        wq=wq_t, wk=wk_t, wv=wv_r, wo=wo_t, w1=w1_t, w2=w2_t,
        bqc=cols(bQf, CT), bkc=cols(bKf, CT), b1c=cols(b1f, FT),
        boeffc=cols(boeff, CT), b2c=cols(b2, CT),
        onescv=np.ones((128, 1), np.float32),
        onesrv=np.ones((1, 128), np.float32),
    )
    masks_by_p = [make_masks(p) for p in range(2)]

    in_maps = []
    for core in range(2 * Bn):
        b, p = core // 2, core % 2
        m = dict(common)
        # own rows: 128-row tiles p, p+2, ... of X[b] (residual read)
        xo = X[b].reshape(n_lt, 128, D)[p::2].reshape(OWN_L, D)
        m["xto"] = np.ascontiguousarray(
            xo.T.reshape(CT, 128, OWN_L).transpose(1, 0, 2))
        # all rows, window-permuted (own tiles first in each 1024-row
        # window), bf16, for replicated full-L LN1/K/V
        xw = X[b].reshape(n_lt // 8, 4, 2, 128, D)  # [win, j, par, r, D]
        xp = np.concatenate([xw[:, :, p], xw[:, :, 1 - p]],
                            axis=1)  # [win, 8, 128, D]
        xp = xp.reshape(L, D)
        m["xtob"] = np.ascontiguousarray(
            xp.T.reshape(CT, 128, L).transpose(1, 0, 2)).astype(BF)
        m["masks"] = masks_by_p[p]
        in_maps.append(m)
    return in_maps


def gather(results, L=L_, D=D_, Bn=B_):
    n_own = (L // 128) // 2
    out = np.empty((Bn, L, D), np.float32)
    for core, r in enumerate(results):
        b, p = core // 2, core % 2
        part = np.ascontiguousarray(r["outT"].T)
        for k in range(n_own):
            out[b, 128 * (p + 2 * k):128 * (p + 2 * k) + 128, :] = \
                part[128 * k:128 * (k + 1), :]
    return out


_NC_CACHE = {}


def get_nc():
    if "nc" not in _NC_CACHE:
        _NC_CACHE["nc"] = build_nc()
    return _NC_CACHE["nc"]


def kernel(**inputs) -> np.ndarray:
    nc, _ = get_nc()
    in_maps = prep_in_maps(inputs)
    res = run_bass_kernel_spmd(nc, in_maps, list(range(N_CORES)))
    return gather(res.results)
